# revision 1
# baseline (speedup 1.0000x reference)
"""Trainium2 Bass kernel for the LSTM decoder (nn_Decoder).

Math (reference):
    u0 = x @ W_u0.T + b_u0
    xi0 = [z, u0, enc]                       # CAT = 64 + 128 + 256 = 448
    h0 = xi0 @ W_h1.T + b_h1 ; c0 = xi0 @ W_h2.T + b_h2
    for t in range(T):
        xi = [z, y_{t-1}, enc]               # y_{-1} = u0, y_t = h_t
        gates = xi @ W_ih.T + h @ W_hh.T + b_ih + b_hh
        i,f,g,o = split(gates); c = sig(f)*c + sig(i)*tanh(g); h = sig(o)*tanh(c)
        y_t = h

Key restructuring:
  * z/enc are time-invariant -> their gate contribution gc = z@Wz.T + enc@We.T + b
    is computed once; per-step matmul is only K=128 (h) after merging
    Wc = W_ih[:, y-cols] + W_hh (valid for t >= 1 where y == h).
  * Layout: hidden (128) on partitions, batch on the free dim. h_t is produced
    directly in the rhs layout the next matmul needs -> zero transposes in loop.
  * Gates reordered [g, f, i, o]: tanh(g) issues right after the first
    recurrent matmul; sigmoid[f|i] spans 2 PSUM banks; sigma(o) is off the
    critical path. [sig_f|sig_i] * [c|tanh_g] is one wide DVE multiply.
  * All loop matmuls in float32r (1 cyc/row vs 4 for fp32); gc injected into
    PSUM by identity-matmuls prefetched one step ahead; inputs packed into
    5 DMA transfers; all cross-engine-read buffers ping-pong by step parity.
  * Data parallel over 8 cores (batch 8192 -> 1024/core); weights replicated.
"""

import sys

sys.path.insert(0, "/opt/trn_rl_repo")

import numpy as np

import concourse.bass as bass  # noqa: F401  (bass must import before bacc)
import concourse.mybir as mybir
import concourse.tile as tile
from concourse import bacc
from concourse.bass_utils import run_bass_kernel_spmd

N_CORES = 8
BS, IN, HID, LAT, OUT = 8192, 48, 256, 64, 128
B = BS // N_CORES  # 1024 batch rows per core
CH = 512           # batch chunk (one PSUM bank per gate tile)
NCH = B // CH      # 2 chunks
F32 = mybir.dt.float32
F32R = mybir.dt.float32r
AF = mybir.ActivationFunctionType

# matmul dtype modes for the scan loop: "f32" (exact, 4 cyc/row) or "f32r"
# (single-pass PE mode, 1 cyc/row at N>=256, slightly different rounding)


# gate reorder: torch order i,f,g,o -> g,f,i,o (g first: its sigmoid feeds the
# DVE 2s-1 fixup early; o last: sigma(o) is off the critical path)
GATE_PERM = np.r_[256:384, 128:256, 0:128, 384:512]

PK64_SPEC = [("zT", 1024), ("WzT", 512), ("Wh1z", 128), ("Wh2z", 128)]
PK128A_SPEC = [("eT0", 1024), ("eT1", 1024), ("WeT0", 512), ("WeT1", 512)]
PK48_SPEC = [("xT", 1024), ("Wu0T", 128)]
PK128B_SPEC = [("WcT", 512), ("WihyT", 512), ("WhhT", 512), ("idm", 128),
               ("Wh1y", 128), ("Wh1e0", 128), ("Wh1e1", 128),
               ("Wh2y", 128), ("Wh2e0", 128), ("Wh2e1", 128)]
PK64_W = sum(w for _, w in PK64_SPEC)
PK128A_W = sum(w for _, w in PK128A_SPEC)
PK48_W = sum(w for _, w in PK48_SPEC)
PK128B_W = sum(w for _, w in PK128B_SPEC)

HALF_SPLIT = False  # split tanh_c/hmul/rec-mm/id-mm into 256-wide halves
ALT_ORDER = False   # alternate chunk emission order per step

_PROGRAM_CACHE: dict = {}


def build_program(n_steps: int, T_out: int):
    """Emit the Bass/Tile program. Runs `n_steps` LSTM steps, writing step t's
    output to y[t % T_out] (n_steps > T_out is used only for timing runs)."""
    nc = bacc.Bacc("TRN2", target_bir_lowering=False, debug=False)

    # packed inputs (one DMA each): see _prep_maps for column layouts
    d_pk64 = nc.dram_tensor("pk64", [LAT, PK64_W], F32R, kind="ExternalInput")
    d_pk128a = nc.dram_tensor("pk128a", [128, PK128A_W], F32R, kind="ExternalInput")
    d_pk48 = nc.dram_tensor("pk48", [IN, PK48_W], F32R, kind="ExternalInput")
    d_pk128b = nc.dram_tensor("pk128b", [128, PK128B_W], F32R, kind="ExternalInput")
    d_bias = nc.dram_tensor("bias7", [128, 7], F32, kind="ExternalInput")
    d_y = nc.dram_tensor("y", [T_out, OUT, B], F32, kind="ExternalOutput")
    y_ap = d_y[:]

    with tile.TileContext(nc) as tc:
        with (
            tc.tile_pool(name="const", bufs=1) as cpool,
            tc.tile_pool(name="state", bufs=1) as spool,
            tc.tile_pool(name="psum", bufs=1, space="PSUM") as ppool,
        ):
            # ---- load packed inputs (ordered by first use) ----
            def load(dram, shape, tag, dt=F32):
                t = cpool.tile(shape, dt, tag=tag, name=tag)
                nc.sync.dma_start(out=t[:], in_=dram[:])
                return t

            pk64 = load(d_pk64, [LAT, PK64_W], "pk64", F32R)
            pk128a = load(d_pk128a, [128, PK128A_W], "pk128a", F32R)
            bias = load(d_bias, [128, 7], "bias")
            pk48 = load(d_pk48, [IN, PK48_W], "pk48", F32R)
            pk128b = load(d_pk128b, [128, PK128B_W], "pk128b", F32R)

            def cols(t, specs):
                out, o = {}, 0
                for nm, w in specs:
                    out[nm] = t[:, o:o + w]
                    o += w
                return out

            c64 = cols(pk64, PK64_SPEC)
            ca = cols(pk128a, PK128A_SPEC)
            c48 = cols(pk48, PK48_SPEC)
            cb = cols(pk128b, PK128B_SPEC)
            zT, WzT = c64["zT"], c64["WzT"]
            Wh1z, Wh2z = c64["Wh1z"], c64["Wh2z"]
            eT0, eT1 = ca["eT0"], ca["eT1"]
            WeT0, WeT1 = ca["WeT0"], ca["WeT1"]
            xT, Wu0T = c48["xT"], c48["Wu0T"]
            WcT, WihyT, WhhT = cb["WcT"], cb["WihyT"], cb["WhhT"]
            idm = cb["idm"]
            Wh = {"Wh1": [Wh1z, cb["Wh1y"], cb["Wh1e0"], cb["Wh1e1"]],
                  "Wh2": [Wh2z, cb["Wh2y"], cb["Wh2e0"], cb["Wh2e1"]]}

            # ---- persistent loop state ----
            gc = spool.tile([128, 4 * B], F32R, tag="gc", name="gc")     # [gate_tile, batch]
            u0 = spool.tile([128, B], F32R, tag="u0", name="u0")
            h0 = spool.tile([128, B], F32R, tag="h0", name="h0")
            sig = [[spool.tile([128, 4 * CH], F32, tag=f"sig{c}{p}", name=f"sig{c}{p}")
                    for p in range(2)] for c in range(NCH)]
            prod = [spool.tile([128, 2 * CH], F32, tag=f"prod{c}", name=f"prod{c}") for c in range(NCH)]
            tcell = [[spool.tile([128, CH], F32, tag=f"tc{c}{p}", name=f"tc{c}{p}")
                      for p in range(2)] for c in range(NCH)]
            # pair[c][p] = [tanh_g | c_cell] ; h ping-pong per chunk
            pair = [[spool.tile([128, 2 * CH], F32, tag=f"pair{c}{p}", name=f"pair{c}{p}") for p in range(2)]
                    for c in range(NCH)]
            hbuf = [[spool.tile([128, CH], F32R, tag=f"h{c}{p}", name=f"h{c}{p}") for p in range(2)]
                    for c in range(NCH)]

            ps = [ppool.tile([128, 2048], F32, tag=f"ps{c}", name=f"ps{c}") for c in range(NCH)]

            MM = nc.tensor.matmul

            # ---- precompute: gc = Wz@z + We@enc + b  (per gate tile) ----
            for c in range(NCH):
                cs = slice(c * CH, (c + 1) * CH)
                for g in range(4):
                    gs = slice(g * 128, (g + 1) * 128)
                    pslice = ps[c][:, g * 512:(g + 1) * 512]
                    MM(pslice, WzT[:, gs], zT[:, cs], start=True, stop=False)
                    MM(pslice, WeT0[:, gs], eT0[:, cs], start=False, stop=False)
                    MM(pslice, WeT1[:, gs], eT1[:, cs], start=False, stop=True)
                    nc.scalar.activation(gc[:, g * B + c * CH: g * B + (c + 1) * CH],
                                         pslice, AF.Identity, bias=bias[:, g:g + 1])

            # ---- precompute: u0, h0, c0 ----
            for c in range(NCH):
                cs = slice(c * CH, (c + 1) * CH)
                pslice = ps[c][:, 0:512]
                MM(pslice, Wu0T[:], xT[:, cs], start=True, stop=True)
                nc.scalar.activation(u0[:, cs], pslice, AF.Identity,
                                     bias=bias[:, 4:5])
            for c in range(NCH):
                cs = slice(c * CH, (c + 1) * CH)
                for W, dst, bcol in ((Wh["Wh1"], h0[:, cs], 5),
                                     (Wh["Wh2"], pair[c][0][:, 0:CH], 6)):
                    pslice = ps[c][:, 512:1024] if bcol == 5 else ps[c][:, 1024:1536]
                    MM(pslice, W[0][:], zT[:, cs], start=True, stop=False)
                    MM(pslice, W[1][:], u0[:, cs], start=False, stop=False)
                    MM(pslice, W[2][:], eT0[:, cs], start=False, stop=False)
                    MM(pslice, W[3][:], eT1[:, cs], start=False, stop=True)
                    nc.scalar.activation(dst, pslice, AF.Identity,
                                         bias=bias[:, bcol:bcol + 1])

            # ---- the scan ----
            def id_mms(t, c):
                # inject gc into the psum banks for step t (start=True resets)
                p = ps[c]
                HS = [(0, 512)] if not HALF_SPLIT else [(0, 256), (256, 256)]
                for g in range(4):
                    for o, w in HS:
                        MM(p[:, g * 512 + o: g * 512 + o + w], idm[:],
                           gc[:, g * B + c * CH + o: g * B + c * CH + o + w],
                           start=True, stop=False)

            for c in range(NCH):
                id_mms(0, c)
            for t in range(n_steps):
                par = t % 2
                corder = range(NCH) if (not ALT_ORDER or t % 2 == 0) \
                    else reversed(range(NCH))
                for c in corder:
                    cs = slice(c * CH, (c + 1) * CH)
                    p = ps[c]

                    def rec_mm(g, o, w):
                        gsl = p[:, g * 512 + o: g * 512 + o + w]
                        wsl = slice(g * 128, (g + 1) * 128)
                        if t == 0:
                            MM(gsl, WihyT[:, wsl], u0[:, cs][:, o:o + w],
                               start=False, stop=False)
                            MM(gsl, WhhT[:, wsl], h0[:, cs][:, o:o + w],
                               start=False, stop=True)
                        else:
                            MM(gsl, WcT[:, wsl],
                               hbuf[c][(t - 1) % 2][:, o:o + w],
                               start=False, stop=True)

                    # pointwise LSTM cell. bank order [g|f|i|o];
                    # sig cols [.|sf|si|so]; pair cols [c|tg]
                    HS = [(0, 512)] if not HALF_SPLIT else [(0, 256), (256, 256)]
                    for o, w in HS:
                        for g in range(4):
                            rec_mm(g, o, w)
                    nc.scalar.activation(pair[c][par][:, CH:2 * CH], p[:, 0:CH],
                                         AF.Tanh)
                    nc.scalar.activation(sig[c][par][:, CH:3 * CH],
                                         p[:, CH:3 * CH], AF.Sigmoid)
                    nc.scalar.activation(sig[c][par][:, 1536:2048],
                                         p[:, 1536:2048], AF.Sigmoid)
                    # banks are consumed; prefetch next step's gc injection
                    if t + 1 < n_steps:
                        id_mms(t + 1, c)
                    nc.vector.tensor_mul(out=prod[c][:], in0=sig[c][par][:, CH:3 * CH],
                                         in1=pair[c][par][:])
                    nc.vector.tensor_add(out=pair[c][1 - par][:, 0:CH],
                                         in0=prod[c][:, 0:CH],
                                         in1=prod[c][:, CH:2 * CH])
                    for o, w in HS:
                        nc.scalar.activation(tcell[c][par][:, o:o + w],
                                             pair[c][1 - par][:, o:o + w], AF.Tanh)
                        nc.vector.tensor_mul(
                            out=hbuf[c][par][:, o:o + w],
                            in0=sig[c][par][:, 1536 + o:1536 + o + w],
                            in1=tcell[c][par][:, o:o + w])
                    nc.sync.dma_start(out=y_ap[t % T_out, :, cs],
                                      in_=hbuf[c][par][:].bitcast(F32))

    nc.finalize()
    return nc


def _prep_maps(x, enc, z, W_ih, W_hh, b_ih, b_hh, W_u0, b_u0, W_h1, b_h1, W_h2,
               b_h2):
    """Host-side weight prep + per-core sharding. Returns list of in_maps."""
    f = lambda a: np.ascontiguousarray(a, dtype=np.float32)
    p = GATE_PERM
    Wc = (W_ih[:, LAT:LAT + OUT] + W_hh)[p]          # [512, 128]
    bias_g = (b_ih + b_hh)[p].reshape(4, 128).T      # [128, 4]
    bias7 = np.concatenate(
        [bias_g, b_u0[:, None], b_h1[:, None], b_h2[:, None]], axis=1)

    parts = {
        "WcT": f(Wc.T),
        "WihyT": f(W_ih[p, LAT:LAT + OUT].T),
        "WhhT": f(W_hh[p].T),
        "WzT": f(W_ih[p, 0:LAT].T),
        "WeT0": f(W_ih[p, LAT + OUT:LAT + OUT + 128].T),
        "WeT1": f(W_ih[p, LAT + OUT + 128:].T),
        "Wu0T": f(W_u0.T),
        "Wh1z": f(W_h1[:, 0:LAT].T), "Wh1y": f(W_h1[:, LAT:LAT + OUT].T),
        "Wh1e0": f(W_h1[:, LAT + OUT:LAT + OUT + 128].T),
        "Wh1e1": f(W_h1[:, LAT + OUT + 128:].T),
        "Wh2z": f(W_h2[:, 0:LAT].T), "Wh2y": f(W_h2[:, LAT:LAT + OUT].T),
        "Wh2e0": f(W_h2[:, LAT + OUT:LAT + OUT + 128].T),
        "Wh2e1": f(W_h2[:, LAT + OUT + 128:].T),
        "idm": f(np.eye(128)),
    }

    def pack(spec, per_core):
        return f(np.concatenate([per_core[nm] if nm in per_core else parts[nm]
                                 for nm, _ in spec], axis=1))

    maps = []
    for core in range(N_CORES):
        rows = slice(core * B, (core + 1) * B)
        pc = {"xT": f(x[rows].T), "zT": f(z[rows].T),
              "eT0": f(enc[rows, 0:128].T), "eT1": f(enc[rows, 128:256].T)}
        maps.append({
            "pk64": pack(PK64_SPEC, pc),
            "pk128a": pack(PK128A_SPEC, pc),
            "pk48": pack(PK48_SPEC, pc),
            "pk128b": pack(PK128B_SPEC, pc),
            "bias7": f(bias7),
        })
    return maps


def run_device(maps, n_steps, T_out):
    key = (n_steps, T_out)
    if key not in _PROGRAM_CACHE:
        _PROGRAM_CACHE[key] = build_program(n_steps, T_out)
    nc = _PROGRAM_CACHE[key]
    return run_bass_kernel_spmd(nc, maps, core_ids=list(range(N_CORES)))


def kernel(x, enc, z, W_ih, W_hh, b_ih, b_hh, W_u0, b_u0, W_h1, b_h1, W_h2, b_h2,
           horizon):
    T = int(horizon)
    maps = _prep_maps(np.asarray(x, np.float32), np.asarray(enc, np.float32),
                      np.asarray(z, np.float32), np.asarray(W_ih, np.float32),
                      np.asarray(W_hh, np.float32), np.asarray(b_ih, np.float32),
                      np.asarray(b_hh, np.float32), np.asarray(W_u0, np.float32),
                      np.asarray(b_u0, np.float32), np.asarray(W_h1, np.float32),
                      np.asarray(b_h1, np.float32), np.asarray(W_h2, np.float32),
                      np.asarray(b_h2, np.float32))
    res = run_device(maps, T, T)
    # device y: [T, OUT, B] per core -> [B, T, 1, OUT], concat over cores
    parts = [r["y"].transpose(2, 0, 1)[:, :, None, :] for r in res.results]
    return np.ascontiguousarray(np.concatenate(parts, axis=0), dtype=np.float32)

